# revision 9
# baseline (speedup 1.0000x reference)
# Trainium2 Bass kernel for nn_DenoisingNet_MLP (LISTA patch-denoiser).
#
# Math (per image, all feature-major / "transposed" layouts, zero transposes):
#   UT[d=64, L]  = unfold(x)            (direct DMA with strided AP from x)
#   lam[1, L]    = MLP(U)               (W4 augmented with b4; b1..b3 via ACT bias)
#   u = max(|lam|, eps); r = 1/u; sig = lam * r / c      (per-token)
#   Ut = U * r/c ;  ytil = Ut @ Dict = y/(c*u)           (in PSUM)
#   soft(v, t) = sign(v) * max(|v| - t, 0)  rescales as  soft(a*u, t*u) = u*soft(a, t),
#   so with ztil = z/u the LISTA chain uses a per-token-constant threshold sig:
#     ztil0 = soft(c*ytil, sig) ;  ztil = soft(ztil @ S + ytil, sig)  x7
#   computed by ONE custom DVE op per 128-atom chunk (reads PSUM, writes SBUF).
#   xp = clip(u * (ztil @ Dict^T), 0, 1)    ->  fold via shifted-identity matmuls
#   out = fold(xp) / coverage               (w cancels in num/den; w != 0 per setup)
#
# Sharding: batch N=8 -> one image per NeuronCore, everything else replicated.

import numpy as np

P = 8
T_ITERS = 7
HIMG = 128
OH = HIMG - P + 1            # 121
L = OH * OH                  # 14641
D = P * P                    # 64
LATOMS = 256
N_CORES = 8
TTOK = 4 * OH                # 484 tokens (4 unfold rows) per tile
EPS = 1e-6

_PROG_CACHE: dict = {}


def _register_soft_op():
    """Register the fused soft-threshold custom DVE op:
    out = sign(w) * relu(|w| - in1),  w = in0 * s0   (s0 per-partition or literal)."""
    from concourse import dve_ops
    from concourse.dve_spec import (
        Spec, Src0, Src1, C0, Zero, Bin, AluOp, relu, select, maxx,
    )

    for op in dve_ops.OPS:
        if op.name == "SOFT_SIGMA_ANT":
            return op

    w = Bin(AluOp.MULTIPLY, Src0, C0)
    aw = maxx(w, Bin(AluOp.SUBTRACT, Zero, w))              # |w|
    r = relu(Bin(AluOp.SUBTRACT, aw, Src1))                 # relu(|w| - sig)
    body = select(Bin(AluOp.IS_LT, w, Zero), Bin(AluOp.SUBTRACT, Zero, r), r)

    def ref(in0, in1, s0, s1, imm2):
        w = in0.astype(np.float32) * s0
        rr = np.maximum(np.abs(w) - in1, 0.0)
        return np.where(w < 0, -rr, rr).astype(np.float32)

    op = dve_ops.DveOp(
        "SOFT_SIGMA_ANT",
        Spec(body=body, reference=ref),
        subdim=False,
        uops_sha={"v3": "3b7913bf082acd88", "v4": "6df987315d6a65a5"},
    )
    dve_ops.OPS.append(op)
    dve_ops._SUB_OPCODE_FOR_NAME[op.name] = (
        dve_ops._CUSTOM_DVE_ROW_BASE + len(dve_ops.OPS) - 1
    )
    assert dve_ops._SUB_OPCODE_FOR_NAME[op.name] < 0x20
    dve_ops.CUSTOM_DVE_SPECS[op.name] = op.spec
    return op


def _host_constants():
    """Shape-only constants shared by all cores."""
    ishift = np.zeros((P, 128, 128), np.float32)
    for di in range(P):
        for k in range(OH):           # lhsT[k, m]: out[m=k+di] += rhs[k]
            ishift[di, k, k + di] = 1.0
    i256 = np.zeros((2, 128, 256), np.float32)
    i256[0, :, 0:128] = np.eye(128, dtype=np.float32)
    i256[1, :, 128:256] = np.eye(128, dtype=np.float32)
    cov = np.zeros((HIMG, HIMG), np.float32)
    for di in range(P):
        for dj in range(P):
            cov[di:di + OH, dj:dj + OH] += 1.0
    covinv = (1.0 / cov).astype(np.float32)
    ones_row = np.ones((1, 128), np.float32)
    return ishift, i256, covinv, ones_row


def _build_program():
    import concourse.bass as bass
    import concourse.mybir as mybir
    from concourse.bacc import Bacc
    from concourse.tile import TileContext
    from contextlib import ExitStack

    soft_op = _register_soft_op()
    fp32 = mybir.dt.float32
    Alu = mybir.AluOpType
    Act = mybir.ActivationFunctionType

    nc = Bacc()

    # --- DRAM I/O ---
    x_d = nc.dram_tensor("x_img", [HIMG, HIMG], fp32, kind="ExternalInput")
    dict_d = nc.dram_tensor("dict_m", [D, LATOMS], fp32, kind="ExternalInput")
    dictt_d = nc.dram_tensor("dict_t", [LATOMS, D], fp32, kind="ExternalInput")
    w1_d = nc.dram_tensor("w1_m", [D, 128], fp32, kind="ExternalInput")
    w2_d = nc.dram_tensor("w2_m", [128, 64], fp32, kind="ExternalInput")
    w3_d = nc.dram_tensor("w3_m", [64, 32], fp32, kind="ExternalInput")
    w4_d = nc.dram_tensor("w4_m", [33, 1], fp32, kind="ExternalInput")  # [W4; b4]
    b1_d = nc.dram_tensor("b1_c", [128, 1], fp32, kind="ExternalInput")
    b2_d = nc.dram_tensor("b2_c", [64, 1], fp32, kind="ExternalInput")
    b3_d = nc.dram_tensor("b3_c", [32, 1], fp32, kind="ExternalInput")
    c_d = nc.dram_tensor("c_col", [128, 1], fp32, kind="ExternalInput")
    ones_d = nc.dram_tensor("ones_r", [1, 128], fp32, kind="ExternalInput")
    ish_d = nc.dram_tensor("ishift", [P, 128, 128], fp32, kind="ExternalInput")
    i256_d = nc.dram_tensor("i256", [2, 128, 256], fp32, kind="ExternalInput")
    cvi_d = nc.dram_tensor("covinv", [128, 128], fp32, kind="ExternalInput")
    xp_d = nc.dram_tensor("xp_scratch", [D, L], fp32)
    out_d = nc.dram_tensor("out_img", [HIMG, HIMG], fp32, kind="ExternalOutput")

    def dap(t, shape):
        ap = []
        stride = 1
        rev = []
        for s in reversed(shape):
            rev.append([stride, s])
            stride *= s
        return bass.AP(t, 0, list(reversed(rev)))

    with TileContext(nc) as tc, ExitStack() as ctx:
        consts = ctx.enter_context(tc.tile_pool(name="consts", bufs=1))
        work = ctx.enter_context(tc.tile_pool(name="work", bufs=3))
        mlp = ctx.enter_context(tc.tile_pool(name="mlp", bufs=2))
        scal = ctx.enter_context(tc.tile_pool(name="scal", bufs=2))
        bcast = ctx.enter_context(tc.tile_pool(name="bcast", bufs=2))
        zp = ctx.enter_context(tc.tile_pool(name="zp", bufs=6))
        xpp = ctx.enter_context(tc.tile_pool(name="xpp", bufs=2))
        gph = ctx.enter_context(tc.tile_pool(name="gph", bufs=2))
        pv = ctx.enter_context(tc.tile_pool(name="pv", bufs=4, space="PSUM"))
        pm = ctx.enter_context(tc.tile_pool(name="pm", bufs=3, space="PSUM"))
        pf = ctx.enter_context(tc.tile_pool(name="pf", bufs=1, space="PSUM"))

        # ---- load constants ----
        dict_sb = consts.tile([D, LATOMS], fp32)
        nc.sync.dma_start(out=dict_sb, in_=dap(dict_d, [D, LATOMS]))
        dictt_sb = [consts.tile([128, D], fp32, name=f"dt{m}") for m in range(2)]
        for m in range(2):
            nc.sync.dma_start(
                out=dictt_sb[m], in_=bass.AP(dictt_d, m * 128 * D, [[D, 128], [1, D]])
            )
        w1_sb = consts.tile([D, 128], fp32)
        nc.sync.dma_start(out=w1_sb, in_=dap(w1_d, [D, 128]))
        w2_sb = consts.tile([128, 64], fp32)
        nc.sync.dma_start(out=w2_sb, in_=dap(w2_d, [128, 64]))
        w3_sb = consts.tile([64, 32], fp32)
        nc.sync.dma_start(out=w3_sb, in_=dap(w3_d, [64, 32]))
        w4_sb = consts.tile([33, 1], fp32)
        nc.sync.dma_start(out=w4_sb, in_=dap(w4_d, [33, 1]))
        b1_sb = consts.tile([128, 1], fp32)
        nc.sync.dma_start(out=b1_sb, in_=dap(b1_d, [128, 1]))
        b2_sb = consts.tile([64, 1], fp32)
        nc.sync.dma_start(out=b2_sb, in_=dap(b2_d, [64, 1]))
        b3_sb = consts.tile([32, 1], fp32)
        nc.sync.dma_start(out=b3_sb, in_=dap(b3_d, [32, 1]))
        c_sb = consts.tile([128, 1], fp32)
        nc.sync.dma_start(out=c_sb, in_=dap(c_d, [128, 1]))
        ones_sb = consts.tile([1, 128], fp32)
        nc.sync.dma_start(out=ones_sb, in_=dap(ones_d, [1, 128]))
        ish_sb = [consts.tile([128, 128], fp32, name=f"ish{i}") for i in range(P)]
        for i in range(P):
            nc.sync.dma_start(
                out=ish_sb[i],
                in_=bass.AP(ish_d, i * 128 * 128, [[128, 128], [1, 128]]),
            )
        i256_sb = [consts.tile([128, 256], fp32, name=f"i256_{i}") for i in range(2)]
        for i in range(2):
            nc.sync.dma_start(
                out=i256_sb[i],
                in_=bass.AP(i256_d, i * 128 * 256, [[256, 128], [1, 256]]),
            )
        cvi_sb = consts.tile([128, 128], fp32)
        nc.sync.dma_start(out=cvi_sb, in_=dap(cvi_d, [128, 128]))

        # cinv = 1/c ;  negcinv = -1/c   (c is a runtime input)
        cinv_sb = consts.tile([128, 1], fp32)
        nc.vector.reciprocal(out=cinv_sb, in_=c_sb)
        negcinv_sb = consts.tile([128, 1], fp32)
        nc.vector.tensor_scalar(
            out=negcinv_sb, in0=cinv_sb, scalar1=-1.0, scalar2=None, op0=Alu.mult
        )

        # ---- S = I - (Dict^T Dict)/c  (symmetric; lhsT chunks S[kc, :]) ----
        s_sb = []
        for kc in range(2):
            g_ps = pm.tile([128, 256], fp32, tag="mps")
            nc.tensor.matmul(
                out=g_ps,
                lhsT=dict_sb[:, kc * 128:(kc + 1) * 128],
                rhs=dict_sb[:, :],
                start=True,
                stop=True,
            )
            s_k = consts.tile([128, 256], fp32, name=f"s_k{kc}")
            nc.vector.tensor_scalar(
                out=s_k, in0=g_ps, scalar1=negcinv_sb[:, 0:1], scalar2=None,
                op0=Alu.mult,
            )
            nc.vector.tensor_add(s_k, s_k, i256_sb[kc])
            s_sb.append(s_k)

        # persistent MLP h3 tile with augmented ones-row (row 32) for b4
        h3_sb = consts.tile([33, TTOK], fp32)
        nc.vector.memset(h3_sb[32:33, :], 1.0)

        # ---- main loop over token tiles ----
        ntiles = (L + TTOK - 1) // TTOK
        for g in range(ntiles):
            t0 = g * TTOK
            tg = min(TTOK, L - t0)
            nr = tg // OH
            i0 = t0 // OH

            ut = work.tile([D, TTOK], fp32, tag="ut")
            for di in range(P):
                nc.sync.dma_start(
                    out=ut[di * P:(di + 1) * P, :tg],
                    in_=bass.AP(
                        x_d, (i0 + di) * HIMG,
                        [[1, P], [HIMG, nr], [1, OH]],
                    ),
                )

            # MLP
            h1_ps = pm.tile([128, tg], fp32, tag="mps")
            nc.tensor.matmul(out=h1_ps, lhsT=w1_sb, rhs=ut[:, :tg], start=True, stop=True)
            h1 = mlp.tile([128, TTOK], fp32, tag="h1")
            nc.scalar.activation(
                out=h1[:, :tg], in_=h1_ps, func=Act.Relu, bias=b1_sb[:, 0:1]
            )
            h2_ps = pm.tile([64, tg], fp32, tag="mps")
            nc.tensor.matmul(out=h2_ps, lhsT=w2_sb, rhs=h1[:, :tg], start=True, stop=True)
            h2 = mlp.tile([64, TTOK], fp32, tag="h2")
            nc.scalar.activation(
                out=h2[:, :tg], in_=h2_ps, func=Act.Relu, bias=b2_sb[:, 0:1]
            )
            h3_ps = pm.tile([32, tg], fp32, tag="mps")
            nc.tensor.matmul(out=h3_ps, lhsT=w3_sb, rhs=h2[:, :tg], start=True, stop=True)
            nc.scalar.activation(
                out=h3_sb[0:32, :tg], in_=h3_ps, func=Act.Relu, bias=b3_sb[:, 0:1]
            )
            lam_ps = pm.tile([1, tg], fp32, tag="mps")
            nc.tensor.matmul(
                out=lam_ps, lhsT=w4_sb, rhs=h3_sb[:, :tg], start=True, stop=True
            )

            # per-token scalars: u = max(|lam|, eps); r ~ 1/u; sig = lam*r/c
            au_sb = scal.tile([1, TTOK], fp32, tag="au")
            nc.scalar.activation(out=au_sb[:, :tg], in_=lam_ps, func=Act.Abs)
            u_sb = scal.tile([1, TTOK], fp32, tag="u")
            nc.vector.tensor_scalar(
                out=u_sb[:, :tg], in0=au_sb[:, :tg], scalar1=EPS, scalar2=None,
                op0=Alu.max,
            )
            r_sb = scal.tile([1, TTOK], fp32, tag="r")
            nc.vector.reciprocal_approx_fast(out=r_sb[:, :tg], in_=u_sb[:, :tg])
            sig_sb = scal.tile([1, TTOK], fp32, tag="sig")
            nc.vector.scalar_tensor_tensor(
                out=sig_sb[:, :tg],
                in0=lam_ps,
                scalar=cinv_sb[0:1, 0:1],
                in1=r_sb[:, :tg],
                op0=Alu.mult,
                op1=Alu.mult,
            )

            # broadcasts via rank-1 matmul (+ ACT copy to SBUF where needed)
            sigb_ps = pm.tile([128, tg], fp32, tag="mps")
            nc.tensor.matmul(
                out=sigb_ps, lhsT=ones_sb[0:1, 0:128], rhs=sig_sb[:, :tg],
                start=True, stop=True,
            )
            sigb = bcast.tile([128, TTOK], fp32, tag="sigb")
            nc.scalar.copy(out=sigb[:, :tg], in_=sigb_ps)

            rb_ps = pm.tile([64, tg], fp32, tag="mps")
            nc.tensor.matmul(
                out=rb_ps, lhsT=ones_sb[0:1, 0:64], rhs=r_sb[:, :tg],
                start=True, stop=True,
            )
            # Ut = U * (r/c)
            utl = work.tile([D, TTOK], fp32, tag="utl")
            nc.vector.scalar_tensor_tensor(
                out=utl[:, :tg],
                in0=ut[:, :tg],
                scalar=cinv_sb[0:64, 0:1],
                in1=rb_ps,
                op0=Alu.mult,
                op1=Alu.mult,
            )

            ub_ps = pm.tile([64, tg], fp32, tag="mps")
            nc.tensor.matmul(
                out=ub_ps, lhsT=ones_sb[0:1, 0:64], rhs=u_sb[:, :tg],
                start=True, stop=True,
            )
            ub = bcast.tile([64, TTOK], fp32, tag="ub")
            nc.scalar.copy(out=ub[:, :tg], in_=ub_ps)

            # ztil0 = soft(c * (Ut @ Dict), sig)
            z_cur = []
            for m in range(2):
                v_ps = pv.tile([128, tg], fp32, tag="v")
                nc.tensor.matmul(
                    out=v_ps,
                    lhsT=dict_sb[:, m * 128:(m + 1) * 128],
                    rhs=utl[:, :tg],
                    start=True,
                    stop=True,
                )
                z_m = zp.tile([128, TTOK], fp32, tag="zt")
                nc.vector._custom_dve(
                    soft_op, out=z_m[:, :tg], in0=v_ps, in1=sigb[:, :tg],
                    s0=c_sb[:, 0:1],
                )
                z_cur.append(z_m)

            # LISTA iterations
            for it in range(T_ITERS):
                z_nxt = []
                for m in range(2):
                    msl = slice(m * 128, (m + 1) * 128)
                    v_ps = pv.tile([128, tg], fp32, tag="v")
                    nc.tensor.matmul(
                        out=v_ps, lhsT=s_sb[0][:, msl], rhs=z_cur[0][:, :tg],
                        start=True, stop=False,
                    )
                    nc.tensor.matmul(
                        out=v_ps, lhsT=s_sb[1][:, msl], rhs=z_cur[1][:, :tg],
                        start=False, stop=False,
                    )
                    nc.tensor.matmul(
                        out=v_ps, lhsT=dict_sb[:, msl], rhs=utl[:, :tg],
                        start=False, stop=True,
                    )
                    z_m = zp.tile([128, TTOK], fp32, tag="zt")
                    nc.vector._custom_dve(
                        soft_op, out=z_m[:, :tg], in0=v_ps, in1=sigb[:, :tg], s0=1.0
                    )
                    z_nxt.append(z_m)
                z_cur = z_nxt

            # xp = clip(u * (ztil @ Dict^T), 0, 1)
            q_ps = pm.tile([64, tg], fp32, tag="mps")
            nc.tensor.matmul(
                out=q_ps, lhsT=dictt_sb[0], rhs=z_cur[0][:, :tg], start=True, stop=False
            )
            nc.tensor.matmul(
                out=q_ps, lhsT=dictt_sb[1], rhs=z_cur[1][:, :tg], start=False, stop=True
            )
            m1 = xpp.tile([D, TTOK], fp32, tag="m1")
            nc.vector.tensor_mul(m1[:, :tg], q_ps, ub[:, :tg])
            xp = xpp.tile([D, TTOK], fp32, tag="xp")
            nc.vector.tensor_scalar(
                out=xp[:, :tg], in0=m1[:, :tg], scalar1=0.0, scalar2=1.0,
                op0=Alu.max, op1=Alu.min,
            )
            nc.sync.dma_start(
                out=bass.AP(xp_d, t0, [[L, D], [1, tg]]), in_=xp[:, :tg]
            )

        # ---- fold: out = (sum of shifted planes) / coverage ----
        fold_ps = pf.tile([128, 128], fp32)
        for di in range(P):
            # plane (di,dj) zero-padded into a 128-wide field at column dj, so
            # every fold matmul writes the full [128,128] PSUM region
            gdi = gph.tile([OH, P, 128], fp32, tag="g")
            nc.gpsimd.memset(gdi, 0.0)
            for dj in range(P):
                nc.sync.dma_start(
                    out=gdi[:, dj, dj:dj + OH],
                    in_=bass.AP(xp_d, (di * P + dj) * L, [[OH, OH], [1, OH]]),
                )
            for dj in range(P):
                nc.tensor.matmul(
                    out=fold_ps,
                    lhsT=ish_sb[di][0:OH, :],
                    rhs=gdi[:, dj, :],
                    start=(di == 0 and dj == 0),
                    stop=(di == P - 1 and dj == P - 1),
                )
        out_sb = consts.tile([128, 128], fp32)
        nc.vector.tensor_mul(out_sb, fold_ps, cvi_sb)
        nc.sync.dma_start(out=dap(out_d, [HIMG, HIMG]), in_=out_sb)

    nc.compile()
    return nc


def _get_program():
    if "nc" not in _PROG_CACHE:
        _PROG_CACHE["nc"] = _build_program()
    return _PROG_CACHE["nc"]


def make_in_maps(x, Dict, c, w, W1, b1, W2, b2, W3, b3, W4, b4):
    ishift, i256, covinv, ones_row = _host_constants()
    f32 = np.float32
    com = {
        "dict_m": np.ascontiguousarray(Dict, f32),
        "dict_t": np.ascontiguousarray(Dict.T, f32),
        "w1_m": np.ascontiguousarray(W1, f32),
        "w2_m": np.ascontiguousarray(W2, f32),
        "w3_m": np.ascontiguousarray(W3, f32),
        "w4_m": np.ascontiguousarray(
            np.concatenate([W4, b4.reshape(1, 1)], axis=0), f32
        ),
        "b1_c": np.ascontiguousarray(b1.reshape(128, 1), f32),
        "b2_c": np.ascontiguousarray(b2.reshape(64, 1), f32),
        "b3_c": np.ascontiguousarray(b3.reshape(32, 1), f32),
        "c_col": np.full((128, 1), np.asarray(c).reshape(-1)[0], f32),
        "ones_r": ones_row,
        "ishift": ishift,
        "i256": i256,
        "covinv": covinv,
    }
    in_maps = []
    for i in range(N_CORES):
        m = dict(com)
        m["x_img"] = np.ascontiguousarray(x[i, 0], f32)
        in_maps.append(m)
    return in_maps


def kernel(**inputs):
    from concourse.bass_utils import run_bass_kernel_spmd

    nc = _get_program()
    in_maps = make_in_maps(**inputs)
    res = run_bass_kernel_spmd(nc, in_maps, core_ids=list(range(N_CORES)))
    out = np.stack([r["out_img"] for r in res.results])[:, None, :, :]
    return out.astype(np.float32)


def _build_floor_program():
    """Tiny passthrough kernel to measure host->device dispatch overhead."""
    import concourse.bass as bass
    import concourse.mybir as mybir
    from concourse.bacc import Bacc
    from concourse.tile import TileContext
    from contextlib import ExitStack

    fp32 = mybir.dt.float32
    nc = Bacc()
    a_d = nc.dram_tensor("a_in", [128, 128], fp32, kind="ExternalInput")
    o_d = nc.dram_tensor("a_out", [128, 128], fp32, kind="ExternalOutput")
    with TileContext(nc) as tc, ExitStack() as ctx:
        pool = ctx.enter_context(tc.tile_pool(name="p", bufs=1))
        t = pool.tile([128, 128], fp32)
        nc.sync.dma_start(out=t, in_=bass.AP(a_d, 0, [[128, 128], [1, 128]]))
        nc.sync.dma_start(out=bass.AP(o_d, 0, [[128, 128], [1, 128]]), in_=t)
    nc.compile()
    return nc


def measure_floor(n=6):
    import time
    from concourse.bass_utils import run_bass_kernel_spmd

    if "floor" not in _PROG_CACHE:
        _PROG_CACHE["floor"] = _build_floor_program()
    nc = _PROG_CACHE["floor"]
    x = np.zeros((128, 128), np.float32)
    maps = [{"a_in": x} for _ in range(N_CORES)]
    ts = []
    for _ in range(n):
        t0 = time.time()
        run_bass_kernel_spmd(nc, maps, core_ids=list(range(N_CORES)))
        ts.append(time.time() - t0)
    return min(ts)
